# revision 5
# baseline (speedup 1.0000x reference)
"""Trainium2 kernel for nn_LSHmodule (sparse_attention), 8 NeuronCores.

Algorithm: the reference runs 64 full dense SxS attentions (one per LSH bucket,
each with one bucket's rows/cols zeroed) and sums them — ~1.1 TFLOP.  That
collapses algebraically to a SINGLE modified attention (~60x fewer FLOPs):

With per-row shift m_s, e[s,t] = exp(sc*q_s.q_t - m_s), bucket one-hot
Bm[t,i], counts cnt_i, outside-mass OM[s,i] = sum_{t not in i} e[s,t],
denominators d[s,i] = OM[s,i] + cnt_i*exp(-m_s), and
r[s,i] = (1/d[s,i]) * [i != bucket(s)]:

    out[s] = sum_t e[s,t] * (sum_i (1-Bm[t,i]) r[s,i]) * v_t
             + (Vtot - V_{bucket(s)}) / S

Sharding: 8 shards = 2 batches x 4 query-row groups of 512 rows.  Each shard
runs on its own NeuronCore via jitted XLA; dispatch is async so all 8 devices
run concurrently.
"""
import sys
sys.path.insert(0, '/opt/trn_rl_repo')
import math
import numpy as np
import jax
import jax.numpy as jnp
from functools import partial

B, S, D = 2, 2048, 512
NB, NH = 64, 6
R = 512                    # rows per shard
SC = 1.0 / math.sqrt(D)


@partial(jax.jit, static_argnums=(), donate_argnums=())
def _shard_fn(x_b, WqT, bq, WvT, bv, hypW, hypB, row0):
    # projections for the full batch (t side)
    q = x_b @ WqT + bq                    # [S, D]
    v = x_b @ WvT + bv                    # [S, D]
    # LSH buckets for all tokens
    proj = q @ hypW + hypB                # [S, NH]
    bits = (proj >= 0).astype(jnp.float32)
    pw = (2.0 ** jnp.arange(NH, dtype=jnp.float32))
    bk = bits @ pw                        # [S]
    ar = jnp.arange(NB, dtype=jnp.float32)
    Bm = (bk[:, None] == ar[None, :]).astype(jnp.float32)   # [S, NB]
    cnt = Bm.sum(0)                       # [NB]

    # own-row slice
    qs = jax.lax.dynamic_slice(q, (row0, 0), (R, D))        # [R, D]
    bks = jax.lax.dynamic_slice(bk, (row0,), (R,))
    Bs = (bks[:, None] == ar[None, :]).astype(jnp.float32)  # [R, NB]

    m = SC * (qs * qs).sum(1)             # [R] diagonal shift (stability)
    e = jnp.exp(SC * (qs @ q.T) - m[:, None])               # [R, S]
    OM = e @ (1.0 - Bm)                   # [R, NB] outside mass (no cancellation)
    d = OM + cnt[None, :] * jnp.exp(-m)[:, None]
    r = (1.0 / jnp.maximum(d, 1e-30)) * (1.0 - Bs)          # own-bucket zeroed
    C = r.sum(1)[:, None] - r @ Bm.T      # [R, S]
    Vtot = v.sum(0)                       # [D]
    Vb = Bm.T @ v                         # [NB, D]
    out = (e * C) @ v + (Vtot[None, :] - Bs @ Vb) * (1.0 / S)
    return out                            # [R, D]


def kernel(x, Wq, bq, Wv, bv, hyperplanes):
    x = np.asarray(x, np.float32)
    WqT = np.ascontiguousarray(np.asarray(Wq, np.float32).T)
    WvT = np.ascontiguousarray(np.asarray(Wv, np.float32).T)
    bq = np.asarray(bq, np.float32)
    bv = np.asarray(bv, np.float32)
    hyp = np.asarray(hyperplanes, np.float32)
    hypW, hypB = hyp[:D], hyp[D]

    devs = jax.devices()[:8]
    futs = []
    for c in range(8):
        b, g = c // 4, c % 4
        dv = devs[c]
        args = [jax.device_put(a, dv) for a in
                (x[b], WqT, bq, WvT, bv, hypW, hypB,
                 np.int32(g * R))]
        futs.append(_shard_fn(*args))      # async dispatch, all 8 run concurrently
    out = np.empty((B, S, D), np.float32)
    for c, f in enumerate(futs):
        b, g = c // 4, c % 4
        out[b, g * R:(g + 1) * R, :] = np.asarray(f)
    return out
